# revision 43
# baseline (speedup 1.0000x reference)
"""KAN (Jacobi/shared) kernel for Trainium2, 8 NeuronCores.

Math: y[b,o,s] = sum_{i,d} P_d(tanh(x[b,i,s])) * C[i,o,d],  P_d = Jacobi(a=1,b=1)
Monomial reformulation (host-side basis change, exact):
  y[b,o,s] = bias[o] + sum_{k=1..4} sum_i t^k[b,i,s] * W_k[i,o],  t = tanh(x)

Device plan (per core, data-parallel over the 65536-point axis):
  - fp16 I/O: x repacked host-side to [64, B*SC] fp16; y written as fp16.
  - Stacked contraction tiles s13 = [t; t^3], s24 = [t^2; t^4] (128 partitions)
    so each output tile needs only 2 K=128 fp16 matmuls (PE cost is
    output-free-size based, so K=128 halves PE time vs K=64).
    Pairing (1,3)/(2,4) keeps every two-input op's sources at partition base 0:
      t   = tanh(x)        -> s13[0:64]    (Act)
      t^2 = t*t            -> s24[0:64]    (DVE)
      t^3 = t*t^2          -> s13[64:128]  (DVE, inputs both base 0)
      t^4 = t^2*t^2        -> s24[64:128]  (Pool/DVE/Act per T4_PAT)
  - PSUM evacuation (fp32 -> fp16 + bias) on Act/DVE per EV_PAT (GPSIMD
    cannot access PSUM).
  - Few large DMAs (the old kernel spent ~83us of SP sequencer issuing 99
    DMAs; this one issues ~20).
"""

import sys

import numpy as np

if "/opt/trn_rl_repo" not in sys.path:
    sys.path.insert(0, "/opt/trn_rl_repo")

B = 4
I = 64
S = 65536
O = 128
NCORES = 8
SC = S // NCORES          # 8192 points per core
F = 4096                  # columns per work chunk
NCH = B * SC // F         # 8 chunks per core (one batch each)
PW = 2048                 # power-op half width
NPS = F // PW             # 2 halves per chunk
EW = 2048                 # psum tile / evac width (4 fp32 banks)
NEV = F // EW             # 2 psum tiles per chunk
MM = 512                  # matmul N (one fp32 psum bank)
PSUM_BUFS = 2

# t^4 engine per 2048-half (16 slots): pool is otherwise idle but slow
# (tensor_tensor runs at 0.42 gpsimd efficiency, ~4.2us/half); keep it off the
# pipeline-fill (first) and drain (last) halves and give DVE a few mid slots
T4_PAT = ["dve", "pool", "pool", "pool", "pool", "dve", "pool", "pool",
          "pool", "pool", "dve", "pool", "pool", "pool", "dve", "dve"]
# evac engine per (chunk, psum-tile); the last two slots (drain chunk) are
# overridden by a parallel Act/DVE split
EV_PAT = ["act", "dve", "act", "act", "act", "dve", "act", "act",
          "act", "act", "act", "dve", "act", "act", "act", "dve"]
WARMUP_MM = 36  # PE p-state warmup matmuls during pipeline fill
TANH_HALVES = False  # split steady-state tanh into 2048-halves

# coeff of t^k (rows) in Jacobi P^(1,1)_d (cols), d=0..4
_MONO = np.array(
    [
        [1.0, 0.0, -0.75, 0.0, 0.625],
        [0.0, 2.0, 0.0, -3.0, 0.0],
        [0.0, 0.0, 3.75, 0.0, -8.75],
        [0.0, 0.0, 0.0, 7.0, 0.0],
        [0.0, 0.0, 0.0, 0.0, 13.125],
    ],
    dtype=np.float64,
)

_CACHE = {}


def _build_nc():
    import concourse.bacc as bacc
    import concourse.tile as tile
    from concourse import mybir

    f32 = mybir.dt.float32
    f16 = mybir.dt.float16

    nc = bacc.Bacc("TRN2", target_bir_lowering=False, debug=False)

    x_dram = nc.dram_tensor("x", [I, B * SC], f16, kind="ExternalInput")
    w13_dram = nc.dram_tensor("w13", [128, O], f16, kind="ExternalInput")
    w24_dram = nc.dram_tensor("w24", [128, O], f16, kind="ExternalInput")
    b_dram = nc.dram_tensor("bias", [O, 1], f32, kind="ExternalInput")
    y_dram = nc.dram_tensor("y", [B, O, SC], f16, kind="ExternalOutput")

    with tile.TileContext(nc) as tc:
        with (
            tc.tile_pool(name="consts", bufs=1) as consts,
            tc.tile_pool(name="xin", bufs=NCH) as xpool,
            tc.tile_pool(name="s13", bufs=5) as s13pool,
            tc.tile_pool(name="s24", bufs=5) as s24pool,
            tc.tile_pool(name="ysb", bufs=2) as ypool,
            tc.tile_pool(name="psum", bufs=PSUM_BUFS, space="PSUM") as pspool,
        ):
            w13 = consts.tile([128, O], f16)
            w24 = consts.tile([128, O], f16)
            bias = consts.tile([O, 1], f32)

            # issue every input DMA upfront so output-DMA waits on the SP
            # sequencer never stall the input stream; first two x chunks go
            # ahead of the consts so the pipeline fill starts ASAP
            xins = []
            for ci in range(NCH):
                xin = xpool.tile([I, F], f16, name="xin")
                xins.append(xin)

            def _xin_dma(ci):
                nc.sync.dma_start(
                    out=xins[ci][:, :], in_=x_dram[:, ci * F : (ci + 1) * F]
                )

            # first chunk's input split in half so its tanh starts sooner
            nc.sync.dma_start(out=xins[0][:, 0:PW], in_=x_dram[:, 0:PW])
            nc.sync.dma_start(out=xins[0][:, PW:F], in_=x_dram[:, PW:F])
            _xin_dma(1)
            nc.sync.dma_start(out=w13[:, :], in_=w13_dram[:, :])
            nc.sync.dma_start(out=w24[:, :], in_=w24_dram[:, :])
            nc.sync.dma_start(out=bias[:, :], in_=b_dram[:, :])
            for ci in range(2, NCH):
                _xin_dma(ci)

            def _emit_t4(ci, h, s24, hs):
                t4 = T4_PAT[ci * NPS + h]
                if t4 == "pool":
                    nc.gpsimd.tensor_mul(s24[I:128, hs], s24[0:I, hs], s24[0:I, hs])
                elif t4 == "dve":
                    nc.vector.tensor_mul(s24[I:128, hs], s24[0:I, hs], s24[0:I, hs])
                else:
                    nc.scalar.activation(
                        s24[I:128, hs], s24[0:I, hs],
                        mybir.ActivationFunctionType.Square,
                    )

            def emit_powers(ci):
                xin = xins[ci]
                s13 = s13pool.tile([128, F], f16, name="s13")
                s24 = s24pool.tile([128, F], f16, name="s24")
                hss = [slice(h * PW, (h + 1) * PW) for h in range(NPS)]
                last = ci == NCH - 1
                if ci == 0:
                    # fill-latency-optimal: finish half 0 end-to-end first
                    for h, hs in enumerate(hss):
                        nc.scalar.activation(
                            s13[0:I, hs], xin[:, hs],
                            mybir.ActivationFunctionType.Tanh,
                        )
                        nc.vector.tensor_mul(s24[0:I, hs], s13[0:I, hs], s13[0:I, hs])
                        nc.vector.tensor_mul(s13[I:128, hs], s13[0:I, hs], s24[0:I, hs])
                        _emit_t4(ci, h, s24, hs)
                    return s13, s24
                if last:
                    # drain chunk: per-half ops, t^3 before t^4 (t^3 feeds the
                    # first matmul of each psum tile)
                    for hs in hss:
                        nc.scalar.activation(
                            s13[0:I, hs], xin[:, hs],
                            mybir.ActivationFunctionType.Tanh,
                        )
                    for hs in hss:
                        nc.vector.tensor_mul(s24[0:I, hs], s13[0:I, hs], s13[0:I, hs])
                    for hs in hss:
                        nc.vector.tensor_mul(s13[I:128, hs], s13[0:I, hs], s24[0:I, hs])
                    for h, hs in enumerate(hss):
                        _emit_t4(ci, h, s24, hs)
                    return s13, s24
                # steady state: full-width ops (fewer per-op overheads);
                # t^4 stays per-half so pool work can be scheduled finely
                if TANH_HALVES:
                    for hs in hss:
                        nc.scalar.activation(
                            s13[0:I, hs], xin[:, hs],
                            mybir.ActivationFunctionType.Tanh,
                        )
                else:
                    nc.scalar.activation(
                        s13[0:I, :], xin[:, :], mybir.ActivationFunctionType.Tanh
                    )
                nc.vector.tensor_mul(s24[0:I, :], s13[0:I, :], s13[0:I, :])
                for h, hs in enumerate(hss):
                    _emit_t4(ci, h, s24, hs)
                nc.vector.tensor_mul(s13[I:128, :], s13[0:I, :], s24[0:I, :])
                return s13, s24

            ev = 0
            ysb = None

            def emit_matmuls(ci, tiles):
                nonlocal ev, ysb
                b, j = divmod(ci, SC // F)
                s13, s24 = tiles
                if j == 0:
                    ysb = ypool.tile([O, SC], f16, name="ysb")
                for h in range(NEV):
                    ps = pspool.tile([O, EW], f32, name="ps")
                    if ci == 0 and h == 0:
                        # PE p-state warmup: harmless overwritten matmuls on
                        # the weight tile while the first chunk's powers are
                        # still being computed (real start=True resets PSUM)
                        for _ in range(WARMUP_MM):
                            nc.tensor.matmul(
                                ps[:, 0:O], w13[:, :], w13[:, :],
                                start=True, stop=True,
                            )
                    # group by stationary weight to avoid per-matmul reloads
                    for q in range(EW // MM):
                        col = h * EW + q * MM
                        nc.tensor.matmul(
                            ps[:, q * MM : (q + 1) * MM],
                            w13[:, :],
                            s13[:, col : col + MM],
                            start=True,
                            stop=False,
                        )
                    for q in range(EW // MM):
                        col = h * EW + q * MM
                        nc.tensor.matmul(
                            ps[:, q * MM : (q + 1) * MM],
                            w24[:, :],
                            s24[:, col : col + MM],
                            start=False,
                            stop=True,
                        )
                    ocol = j * F + h * EW
                    if ci == NCH - 1:
                        # drain chunk: split each evac across Act and DVE so
                        # the final psum drains in parallel
                        hw = EW // 2
                        nc.scalar.activation(
                            ysb[:, ocol : ocol + hw],
                            ps[:, 0:hw],
                            mybir.ActivationFunctionType.Identity,
                            bias=bias[:, 0:1],
                        )
                        nc.vector.tensor_scalar_add(
                            ysb[:, ocol + hw : ocol + EW], ps[:, hw:EW],
                            bias[:, 0:1],
                        )
                    elif EV_PAT[ev] == "act":
                        nc.scalar.activation(
                            ysb[:, ocol : ocol + EW],
                            ps[:, :],
                            mybir.ActivationFunctionType.Identity,
                            bias=bias[:, 0:1],
                        )
                    else:
                        nc.vector.tensor_scalar_add(
                            ysb[:, ocol : ocol + EW], ps[:, :], bias[:, 0:1]
                        )
                    ev += 1
                if j == SC // F - 1:
                    # output DMAs: halves, but finer pieces for the last batch
                    # to shorten the drain tail
                    if b < B - 1:
                        bounds = [0, SC // 2, SC]
                    else:
                        bounds = [0, 2048, 4096, 5120, 6144, 7168, 8192]
                    for lo, hi in zip(bounds[:-1], bounds[1:]):
                        nc.sync.dma_start(out=y_dram[b, :, lo:hi], in_=ysb[:, lo:hi])

            # one-chunk software pipeline: emit powers(k) before matmuls+evac
            # of (k-1) so in-order engine sequencers never head-of-line block
            # the next chunk's elementwise work behind an evac that waits on
            # matmuls
            prev = None
            for ci in range(NCH):
                tiles = emit_powers(ci)
                if prev is not None:
                    emit_matmuls(ci - 1, prev)
                prev = tiles
            emit_matmuls(NCH - 1, prev)
    nc.compile()
    return nc


def _get_nc():
    if "nc" not in _CACHE:
        _CACHE["nc"] = _build_nc()
    return _CACHE["nc"]


def _host_weights(jacobi_coeffs: np.ndarray):
    c = jacobi_coeffs.astype(np.float64)  # (I, O, 5)
    cm = np.einsum("iod,kd->iok", c, _MONO)  # monomial coords, k=0..4
    bias = cm[:, :, 0].sum(axis=0).astype(np.float32).reshape(O, 1)
    w13 = np.concatenate([cm[:, :, 1], cm[:, :, 3]], axis=0).astype(np.float16)
    w24 = np.concatenate([cm[:, :, 2], cm[:, :, 4]], axis=0).astype(np.float16)
    return np.ascontiguousarray(w13), np.ascontiguousarray(w24), bias


def kernel(x: np.ndarray, jacobi_coeffs: np.ndarray) -> np.ndarray:
    from concourse.bass_utils import run_bass_kernel_spmd

    w13, w24, bias = _host_weights(np.asarray(jacobi_coeffs))
    x = np.asarray(x)

    in_maps = []
    for c in range(NCORES):
        xc = x[:, :, c * SC : (c + 1) * SC]  # (B, I, SC)
        x64 = np.ascontiguousarray(
            xc.transpose(1, 0, 2).reshape(I, B * SC).astype(np.float16)
        )
        in_maps.append({"x": x64, "w13": w13, "w24": w24, "bias": bias})

    res = run_bass_kernel_spmd(_get_nc(), in_maps, core_ids=list(range(NCORES)))
    y = np.concatenate([r["y"] for r in res.results], axis=2)
    return np.ascontiguousarray(y.astype(np.float32))
